# revision 17
# baseline (speedup 1.0000x reference)
"""Bass/Tile TRN2 kernel for nn_Disen_GAT_For_Multi_Aspect (v3).

Contract: kernel(**inputs) takes FULL fp32 numpy inputs (keys as in
reference.setup_inputs()) and returns the FULL [B, A, H] fp32 output.

Strategy
--------
Data-parallel over batch B across the 8 cores (1 batch row / core, A=4
aspects per core).  The reference collapses algebraically:

  q = Wq^T asp + bq;  u = TA q; v = TB q; y = W1b v; a3 = W1a^T q
  w[k] = sum_{i,j} q_i v_j T1[i,j,k]
  G = Wk @ [q|w|y|u]   (per aspect, 4 vectors in D-space)
  logits: ch0 = (t.Gq + Cb)/S, ch1 = (x.Gq + Cb)/S,
          ch2 = (x.Gw + x.Gy + d.Gu + Cdw)/S
  Cb = q.bk;  Cdw = bk.(u+w+y) + (a3 + W1_b).v + trans_b.q
  att = sum_ch comb_w[ch] * softmax_masked(logit_ch)
  att_z[h] = sum_n att_n (Wv^T x_n + bv)_h (Wv^T t_n + bv)_h

v3: ALL aspect-level math (q/u/v/y/w/G, the T1 tensor contraction, the
scalar bias terms) is precomputed on the host in fp64 - it is <1% of
the FLOPs but was ~17us of PE time and 2.1MB of T1 DMA.  The device
only does the stream work per aspect:
 * V matmuls (bf16): V_W = Wv^T X, V_T = Wv^T T  ([128, 512] each)
 * row logits into one PSUM bank via tile_position quadrants:
   Dp-rows@0 (G.u vs Dp as fp8 DoubleRow: 2 K-chunks per instruction;
   the ISA requires dst partition 0 for DoubleRow), X-rows@32
   (G.q/w/y vs X), T-rows@64 (G.q vs T).
 * softmax: combo matmul [97->3] (partition 96 holds a persistent
   -1e30*(1-mask) row) -> ACT Exp(bias, scale, accum z) -> reciprocal
   -> alpha broadcast matmul -> two fused vector ops for
   attz = sum_n att*(VW+bv)*(VT+bv)  (scalar_tensor_tensor accum).
"""

import contextlib
import ctypes
import sys
import types

import numpy as np
import ml_dtypes

import concourse.bacc as bacc
import concourse.mybir as mybir
import concourse.tile as tile
from concourse.bass_utils import run_bass_kernel_spmd

B, A, N, D, H = 8, 4, 512, 1024, 128
SCALE = float(np.sqrt(H))
NCORES = 8
DC = D // H  # 8 contraction chunks of 128
GW = 48      # gall panel width (4 cols per aspect + 32 zero pad)
G8S = 64.0   # fp8 scale for the Dp lhsT panel

F32 = mybir.dt.float32
BF16 = mybir.dt.bfloat16
F8 = mybir.dt.float8e4
BF = ml_dtypes.bfloat16
E4 = ml_dtypes.float8_e4m3fn
AF = mybir.ActivationFunctionType
OP = mybir.AluOpType
DR = mybir.MatmulPerfMode.DoubleRow

# cpackf (f32) column layout
CF_BV = 0              # bv column
CF_BA = 1              # bias_all [3 partitions, 4 cols]
CF_CW = 5              # comb_w column (3 partitions)
CF_W = 6
# cpackb (bf16) column layout (gall + combo; Wv ships separately so
# the first V matmul's weights land before the big stream DMAs)
CB_GALL = 0            # [128, 8, 48] G panel chunk-packed
CB_COMBO = DC * GW     # [97, 3] combo matrix
CB_W = DC * GW + 3

NWARM = 6

LAST_RESULTS = None  # test harness peeks at this


def _build(ncores=NCORES):
    nc = bacc.Bacc("TRN2", target_bir_lowering=False, debug=False,
                   num_devices=ncores)

    xs = nc.dram_tensor("xs", [A, 128, 2, DC, N], BF16, kind="ExternalInput")
    dp8 = nc.dram_tensor("dp8", [A, 128, DC, N], F8, kind="ExternalInput")
    cpackf = nc.dram_tensor("cpackf", [128, CF_W], F32, kind="ExternalInput")
    wvpk = nc.dram_tensor("wvpk", [128, DC * H], BF16, kind="ExternalInput")
    cpackb = nc.dram_tensor("cpackb", [128, CB_W], BF16, kind="ExternalInput")
    gal8 = nc.dram_tensor("gal8", [128, DC * GW], F8, kind="ExternalInput")
    ngrow = nc.dram_tensor("ngrow", [1, N], BF16, kind="ExternalInput")
    out = nc.dram_tensor("out", [H, A], F32, kind="ExternalOutput")

    inv_s = 1.0 / SCALE

    with tile.TileContext(nc) as tc:
        with (
            tc.tile_pool(name="const", bufs=1) as cp,
            tc.tile_pool(name="xzone", bufs=4) as xp,
            tc.tile_pool(name="work", bufs=2) as wp,
            tc.tile_pool(name="vzone", bufs=4, space="PSUM") as vps,
            tc.tile_pool(name="rzone", bufs=2, space="PSUM") as rps,
            tc.tile_pool(name="szone", bufs=2, space="PSUM") as sps,
        ):
            # ---- PE warm-up: opens the clock gate before real work ----
            wuc = cp.tile([128, 1], BF16, tag="wuc")
            nc.vector.memset(wuc, 1.0)
            wub = cp.tile([128, N], BF16, tag="wub")
            nc.vector.memset(wub, 1.0)
            ps_wu = sps.tile([1, N], F32, tag="s")
            for i in range(NWARM):
                nc.tensor.matmul(ps_wu, lhsT=wuc, rhs=wub,
                                 start=(i == 0), stop=(i == NWARM - 1))

            # ---- input DMAs (all up-front; tiles are per-aspect) ------
            wvs = cp.tile([128, DC, H], BF16, tag="wvs")
            nc.sync.dma_start(out=wvs.rearrange("p c h -> p (c h)"),
                              in_=wvpk.ap())
            cpf = cp.tile([128, CF_W], F32, tag="cpf")
            nc.scalar.dma_start(out=cpf, in_=cpackf.ap())
            cpb = cp.tile([128, CB_W], BF16, tag="cpb")
            g8 = cp.tile([128, DC, GW], F8, tag="g8")
            rows_bf = []
            for i in range(2):
                rb = cp.tile([97, N], BF16, tag=f"rows{i}")
                rows_bf.append(rb)

            xx_t, xt_t, dp_t = {}, {}, {}
            for a in range(A):
                xx = xp.tile([128, DC, N], BF16, tag="xx")
                if a == 0:  # quarters for earliest first-chunk arrival
                    for i in range(4):
                        eng = nc.scalar if i % 2 == 0 else nc.sync
                        eng.dma_start(out=xx[:, 2 * i:2 * i + 2],
                                      in_=xs.ap()[a, :, 0, 2 * i:2 * i + 2])
                    # gall panel needed from aspect-0 X-rows on
                    nc.scalar.dma_start(out=cpb, in_=cpackb.ap())
                else:
                    nc.sync.dma_start(out=xx[:, 0:DC // 2],
                                      in_=xs.ap()[a, :, 0, 0:DC // 2])
                    nc.scalar.dma_start(out=xx[:, DC // 2:DC],
                                        in_=xs.ap()[a, :, 0, DC // 2:DC])
                dpa = xp.tile([128, DC, N], F8, tag="dp")
                nc.gpsimd.dma_start(out=dpa, in_=dp8.ap()[a])
                xt_ = xp.tile([128, DC, N], BF16, tag="xt")
                nc.sync.dma_start(out=xt_[:, 0:DC // 2],
                                  in_=xs.ap()[a, :, 1, 0:DC // 2])
                nc.scalar.dma_start(out=xt_[:, DC // 2:DC],
                                    in_=xs.ap()[a, :, 1, DC // 2:DC])
                xx_t[a], xt_t[a], dp_t[a] = xx, xt_, dpa
                if a == 0:  # small consts, needed from aspect-0 Dp/combo on
                    nc.gpsimd.dma_start(
                        out=g8.rearrange("p c g -> p (c g)"), in_=gal8.ap())
                    for rb in rows_bf:
                        nc.gpsimd.dma_start(out=rb[96:97, :], in_=ngrow.ap())

            # ---- constant views ---------------------------------------
            wv_v = wvs
            gall_v = cpb[:, CB_GALL:CB_GALL + DC * GW].rearrange(
                "p (c g) -> p c g", c=DC)
            combo_m = cpb[0:97, CB_COMBO:CB_COMBO + 3]
            bv_c = cpf[:, CF_BV:CF_BV + 1]
            bias_all = cpf[0:3, CF_BA:CF_BA + A]
            combw3 = cpf[0:3, CF_CW:CF_CW + 1]

            ones3r = cp.tile([3, 128], BF16, tag="ones3r")
            nc.vector.memset(ones3r, 1.0)
            attz = cp.tile([H, A], F32, tag="attz")

            # ---- per-aspect stream work -------------------------------
            # Uniform-config matmul chains run at 216ns/instr; alternating
            # PE tile configs cost ~+105ns each, so chains stay contiguous.
            for a in range(A):
                xx, xt_, da = xx_t[a], xt_t[a], dp_t[a]
                win = slice(4 * a, 4 * a + 32)

                ps_vw = vps.tile([H, N], F32, tag="v")
                for c in range(DC):
                    nc.tensor.matmul(ps_vw, lhsT=wv_v[:, c, :],
                                     rhs=xx[:, c, :], start=(c == 0),
                                     stop=(c == DC - 1))
                ps_rows = rps.tile([128, N], F32, tag="rows")
                for c in range(DC):
                    nc.tensor.matmul(ps_rows[32:64, :],
                                     lhsT=gall_v[:, c, win],
                                     rhs=xx[:, c, :], start=(c == 0),
                                     stop=(c == DC - 1),
                                     tile_position=(0, 32))
                # Dp rows: fp8 DoubleRow, 2 K-chunks per instruction
                for c2 in range(DC // 2):
                    nc.tensor.matmul(ps_rows[0:32, :],
                                     lhsT=g8[:, 2 * c2:2 * c2 + 2, win],
                                     rhs=da[:, 2 * c2:2 * c2 + 2, :],
                                     start=(c2 == 0), stop=(c2 == DC // 2 - 1),
                                     perf_mode=DR, tile_position=(0, 0))
                # V_W epilogue on scalar while Dp/T rows stream; also cast
                # the finished Dp+X row quadrants so only the T quadrant
                # remains after the T-row chain
                vvw = wp.tile([H, N], BF16, tag="vvw")
                nc.scalar.activation(vvw, ps_vw, AF.Identity, bias=bv_c)
                rb = rows_bf[a % 2]
                nc.scalar.activation(rb[0:64, :], ps_rows[0:64, :],
                                     AF.Identity)
                # T rows
                for c in range(DC):
                    nc.tensor.matmul(ps_rows[64:96, :],
                                     lhsT=gall_v[:, c, win],
                                     rhs=xt_[:, c, :], start=(c == 0),
                                     stop=(c == DC - 1),
                                     tile_position=(0, 64))
                nc.scalar.activation(rb[64:96, :], ps_rows[64:96, :],
                                     AF.Identity)
                # V_T chain, with the combo matmul slotted in after chunk 1
                # so the softmax scalar/vector chain overlaps the rest
                ps_vt = vps.tile([H, N], F32, tag="v")
                ps_combo = sps.tile([3, N], F32, tag="s")
                e3 = wp.tile([3, N], BF16, tag="e3")
                z3 = wp.tile([3, 1], F32, tag="z3")
                for c in range(DC):
                    nc.tensor.matmul(ps_vt, lhsT=wv_v[:, c, :],
                                     rhs=xt_[:, c, :], start=(c == 0),
                                     stop=(c == DC - 1))
                    if c == 1:
                        nc.tensor.matmul(ps_combo, lhsT=combo_m, rhs=rb,
                                         start=True, stop=True)
                        nc.scalar.activation(e3, ps_combo, AF.Exp,
                                             bias=bias_all[:, a:a + 1],
                                             scale=inv_s, accum_out=z3)
                rz = wp.tile([3, 1], F32, tag="rz")
                nc.vector.reciprocal(rz, z3)
                alpha = wp.tile([3, 1], F32, tag="alpha")
                nc.vector.tensor_mul(alpha, rz, combw3)
                arep = wp.tile([3, H], BF16, tag="arep")
                nc.vector.tensor_scalar_mul(arep, ones3r, alpha)
                # pprod = (VT+bv)*(VW+bv) right after the VT chain, then
                # attMM, then one multiply-accumulate into attz[:, a]
                pprod = wp.tile([H, N], BF16, tag="pprod")
                nc.vector.scalar_tensor_tensor(
                    pprod, ps_vt, bv_c, vvw, op0=OP.add, op1=OP.mult)
                ps_att = sps.tile([H, N], F32, tag="s")
                nc.tensor.matmul(ps_att, lhsT=arep, rhs=e3,
                                 start=True, stop=True)
                junk = wp.tile([H, N], BF16, tag="junk")
                nc.vector.scalar_tensor_tensor(
                    junk, ps_att, 1.0, pprod, op0=OP.mult, op1=OP.mult,
                    accum_out=attz[:, a:a + 1])

            nc.sync.dma_start(out=out.ap(), in_=attz)

    nc.compile()
    return nc


def _host_precompute(f):
    """All aspect-level math in fp64 on host -> per-core const packs."""
    S = SCALE
    Wq = f["Wq"].astype(np.float64)
    Wk = f["Wk"].astype(np.float64)
    TA = f["trans_W"][:H].astype(np.float64)   # [H, H]
    TB = f["trans_W"][H:].astype(np.float64)
    W1a = f["W1_W"][:H].astype(np.float64)
    W1b = f["W1_W"][H:].astype(np.float64)
    T1 = f["T1"].astype(np.float64)
    bq, bk = f["bq"].astype(np.float64), f["bk"].astype(np.float64)
    W1_b = f["W1_b"].astype(np.float64)
    trans_b = f["trans_b"].astype(np.float64)

    asp = f["aspect_feature"].astype(np.float64)          # [B, A, D]
    q = asp @ Wq + bq                                     # [B, A, H]
    u = np.einsum("kh,bah->bak", TA, q)
    v = np.einsum("jh,bah->baj", TB, q)
    y = np.einsum("kj,baj->bak", W1b, v)
    a3 = np.einsum("ij,bai->baj", W1a, q)
    QT = np.einsum("bai,ijk->bajk", q, T1)
    w = np.einsum("bajk,baj->bak", QT, v)
    G = np.stack([np.einsum("dh,bah->bad", Wk, t) for t in (q, w, y, u)],
                 axis=-1)                                 # [B, A, D, 4]
    Cb = q @ bk                                           # [B, A]
    Cdw = ((u + w + y) @ bk + ((a3 + W1_b) * v).sum(-1) + q @ trans_b)
    bias_all = np.stack([Cb, Cb, Cdw], axis=1) / S        # [B, 3, A]
    return G, bias_all


def _prep_inputs(inputs):
    f = {k: np.asarray(v, dtype=np.float32) for k, v in inputs.items()}
    G, bias_all = _host_precompute(f)

    wvpk = np.transpose(
        f["Wv"].reshape(DC, 128, H), (1, 0, 2)).reshape(128, DC * H)
    cpackb = np.zeros((128, CB_W), np.float32)
    # quadrants: Dp@0, X@32, T@64, neg@96.
    # combo rows: ch0(TW): st@64+neg; ch1(Wi): sxq@32+neg;
    # ch2(DW): sxw@33, sxy@34, sd@3 (fp8 panel scaled by G8S), neg
    cpackb[64, CB_COMBO + 0] = 1.0
    cpackb[96, CB_COMBO + 0] = 1.0
    cpackb[32, CB_COMBO + 1] = 1.0
    cpackb[96, CB_COMBO + 1] = 1.0
    cpackb[33, CB_COMBO + 2] = 1.0
    cpackb[34, CB_COMBO + 2] = 1.0
    cpackb[3, CB_COMBO + 2] = 1.0 / G8S
    cpackb[96, CB_COMBO + 2] = 1.0

    in_maps = []
    for b in range(NCORES):
        # gall panel [128, DC, 48]: cols 4a+s = G[b, a, :, s]
        gp = np.zeros((D, GW), np.float64)
        for a in range(A):
            gp[:, 4 * a:4 * a + 4] = G[b, a]
        gp = gp.reshape(DC, 128, GW).transpose(1, 0, 2)   # [128, DC, GW]
        cb = cpackb.copy()
        cb[:, CB_GALL:CB_GALL + DC * GW] = gp.reshape(128, DC * GW)

        cf = np.zeros((128, CF_W), np.float32)
        cf[:, CF_BV] = f["bv"]
        cf[0:3, CF_BA:CF_BA + A] = bias_all[b]
        cf[0:3, CF_CW] = f["comb_w"]

        m = {
            "cpackb": cb.astype(BF),
            "wvpk": wvpk.astype(BF),
            "cpackf": cf,
            "gal8": np.clip(gp * G8S, -448, 448).astype(E4).reshape(
                128, DC * GW),
            "ngrow": (-1e30 * (1.0 - f["fmask"][b]))[None, :].astype(BF),
        }
        xst = np.stack([f["feature"][b], f["all_type_feature"][b]], axis=1)
        # [A, 2, N, D] -> [A, 128(p), 2(s), DC(c), N]
        m["xs"] = np.ascontiguousarray(
            xst.transpose(0, 1, 3, 2).reshape(A, 2, DC, 128, N)
               .transpose(0, 3, 1, 2, 4)).astype(BF)
        dpt = f["dep_feature"][b].transpose(0, 2, 1).reshape(A, DC, 128, N)
        m["dp8"] = np.clip(np.ascontiguousarray(dpt.transpose(0, 2, 1, 3)),
                           -240, 240).astype(E4)
        in_maps.append(m)
    return in_maps


def _install_ntff_shim():
    """Provide antenv.axon_hooks (absent in this image) so trace=True can
    drive NTFF capture through libaxon_pjrt.so."""
    if "antenv.axon_hooks" in sys.modules:
        return
    import antenv

    mod = types.ModuleType("antenv.axon_hooks")
    mod._hook = None
    mod.set_axon_ntff_profile_hook = lambda h: setattr(mod, "_hook", h)
    mod.get_axon_ntff_profile_hook = lambda: mod._hook
    sys.modules["antenv.axon_hooks"] = mod
    antenv.axon_hooks = mod

    so_path = "/opt/axon/libaxon_pjrt.so"
    try:
        lib = ctypes.CDLL(so_path)
    except OSError:
        return
    if not hasattr(lib, "axon_start_nrt_profile"):
        return
    lib.axon_start_nrt_profile.argtypes = [ctypes.POINTER(ctypes.c_int64),
                                           ctypes.c_size_t]
    lib.axon_start_nrt_profile.restype = ctypes.c_int64
    lib.axon_stop_nrt_profile.argtypes = [ctypes.c_char_p]
    lib.axon_stop_nrt_profile.restype = ctypes.c_int64

    @contextlib.contextmanager
    def _hook(output_dir, device_ids):
        import jax

        jax.devices()
        if device_ids:
            ids = (ctypes.c_int64 * len(device_ids))(*device_ids)
            rc = lib.axon_start_nrt_profile(ids, len(device_ids))
        else:
            rc = lib.axon_start_nrt_profile(None, 0)
        if rc != 0:
            raise RuntimeError(f"axon_start_nrt_profile rc={rc}")
        try:
            yield
        finally:
            n = lib.axon_stop_nrt_profile(str(output_dir).encode())
            print(f"profile: {n} file(s) written to {output_dir}")

    mod.set_axon_ntff_profile_hook(_hook)


def kernel(feature, dep_feature, aspect_feature, all_type_feature, fmask,
           Wq, bq, Wk, bk, Wv, bv, trans_W, trans_b, T1, W1_W, W1_b, comb_w,
           _profile=False, _tmpdir=None):
    global LAST_RESULTS
    inputs = dict(feature=feature, dep_feature=dep_feature,
                  aspect_feature=aspect_feature,
                  all_type_feature=all_type_feature, fmask=fmask, Wq=Wq,
                  bq=bq, Wk=Wk, bk=bk, Wv=Wv, bv=bv, trans_W=trans_W,
                  trans_b=trans_b, T1=T1, W1_W=W1_W, W1_b=W1_b,
                  comb_w=comb_w)
    nc = _build()
    in_maps = _prep_inputs(inputs)
    if _profile:
        _install_ntff_shim()
    res = run_bass_kernel_spmd(nc, in_maps, list(range(NCORES)),
                               trace=_profile, tmpdir=_tmpdir)
    LAST_RESULTS = res
    full = np.stack([res.results[c]["out"].T for c in range(NCORES)])
    return full.astype(np.float32)


# revision 19
# speedup vs baseline: 1.0233x; 1.0233x over previous
"""Bass/Tile TRN2 kernel for nn_Disen_GAT_For_Multi_Aspect (v3).

Contract: kernel(**inputs) takes FULL fp32 numpy inputs (keys as in
reference.setup_inputs()) and returns the FULL [B, A, H] fp32 output.

Strategy
--------
Data-parallel over batch B across the 8 cores (1 batch row / core, A=4
aspects per core).  The reference collapses algebraically:

  q = Wq^T asp + bq;  u = TA q; v = TB q; y = W1b v; a3 = W1a^T q
  w[k] = sum_{i,j} q_i v_j T1[i,j,k]
  G = Wk @ [q|w|y|u]   (per aspect, 4 vectors in D-space)
  logits: ch0 = (t.Gq + Cb)/S, ch1 = (x.Gq + Cb)/S,
          ch2 = (x.Gw + x.Gy + d.Gu + Cdw)/S
  Cb = q.bk;  Cdw = bk.(u+w+y) + (a3 + W1_b).v + trans_b.q
  att = sum_ch comb_w[ch] * softmax_masked(logit_ch)
  att_z[h] = sum_n att_n (Wv^T x_n + bv)_h (Wv^T t_n + bv)_h

v3: ALL aspect-level math (q/u/v/y/w/G, the T1 tensor contraction, the
scalar bias terms) is precomputed on the host in fp64 - it is <1% of
the FLOPs but was ~17us of PE time and 2.1MB of T1 DMA.  The device
only does the stream work per aspect:
 * V matmuls (bf16): V_W = Wv^T X, V_T = Wv^T T  ([128, 512] each)
 * row logits into one PSUM bank via tile_position quadrants:
   Dp-rows@0 (G.u vs Dp as fp8 DoubleRow: 2 K-chunks per instruction;
   the ISA requires dst partition 0 for DoubleRow), X-rows@32
   (G.q/w/y vs X), T-rows@64 (G.q vs T).
 * softmax: combo matmul [97->3] (partition 96 holds a persistent
   -1e30*(1-mask) row) -> ACT Exp(bias, scale, accum z) -> reciprocal
   -> alpha broadcast matmul -> two fused vector ops for
   attz = sum_n att*(VW+bv)*(VT+bv)  (scalar_tensor_tensor accum).
"""

import contextlib
import ctypes
import sys
import types

import numpy as np
import ml_dtypes

import concourse.bacc as bacc
import concourse.mybir as mybir
import concourse.tile as tile
from concourse.bass_utils import run_bass_kernel_spmd

B, A, N, D, H = 8, 4, 512, 1024, 128
SCALE = float(np.sqrt(H))
NCORES = 8
DC = D // H  # 8 contraction chunks of 128
GW = 48      # gall panel width (4 cols per aspect + 32 zero pad)
G8S = 64.0   # fp8 scale for the Dp lhsT panel

F32 = mybir.dt.float32
BF16 = mybir.dt.bfloat16
F8 = mybir.dt.float8e4
BF = ml_dtypes.bfloat16
E4 = ml_dtypes.float8_e4m3fn
AF = mybir.ActivationFunctionType
OP = mybir.AluOpType
DR = mybir.MatmulPerfMode.DoubleRow

# cpackf (f32) column layout
CF_BV = 0              # bv column
CF_BA = 1              # bias_all [3 partitions, 4 cols]
CF_CW = 5              # comb_w column (3 partitions)
CF_W = 6
# cpackb (bf16) column layout (gall + combo; Wv ships separately so
# the first V matmul's weights land before the big stream DMAs)
CB_GALL = 0            # [128, 8, 48] G panel chunk-packed
CB_COMBO = DC * GW     # [97, 3] combo matrix
CB_W = DC * GW + 3

NWARM = 6

LAST_RESULTS = None  # test harness peeks at this


def _build(ncores=NCORES):
    nc = bacc.Bacc("TRN2", target_bir_lowering=False, debug=False,
                   num_devices=ncores)

    xs = nc.dram_tensor("xs", [A, 128, 2, DC, N], BF16, kind="ExternalInput")
    dp8 = nc.dram_tensor("dp8", [A, 128, DC, N], F8, kind="ExternalInput")
    cpackf = nc.dram_tensor("cpackf", [128, CF_W], F32, kind="ExternalInput")
    wvpk = nc.dram_tensor("wvpk", [128, DC * H], BF16, kind="ExternalInput")
    cpackb = nc.dram_tensor("cpackb", [128, CB_W], BF16, kind="ExternalInput")
    gal8 = nc.dram_tensor("gal8", [128, DC * GW], F8, kind="ExternalInput")
    ngrow = nc.dram_tensor("ngrow", [1, N], BF16, kind="ExternalInput")
    out = nc.dram_tensor("out", [H, A], F32, kind="ExternalOutput")

    inv_s = 1.0 / SCALE

    with tile.TileContext(nc) as tc:
        with (
            tc.tile_pool(name="const", bufs=1) as cp,
            tc.tile_pool(name="xzone", bufs=4) as xp,
            tc.tile_pool(name="work", bufs=2) as wp,
            tc.tile_pool(name="vzone", bufs=4, space="PSUM") as vps,
            tc.tile_pool(name="rzone", bufs=2, space="PSUM") as rps,
            tc.tile_pool(name="szone", bufs=2, space="PSUM") as sps,
        ):
            # ---- PE warm-up: opens the clock gate before real work ----
            wuc = cp.tile([128, 1], BF16, tag="wuc")
            nc.vector.memset(wuc, 1.0)
            wub = cp.tile([128, N], BF16, tag="wub")
            nc.vector.memset(wub, 1.0)
            ps_wu = sps.tile([1, N], F32, tag="s")
            for i in range(NWARM):
                nc.tensor.matmul(ps_wu, lhsT=wuc, rhs=wub,
                                 start=(i == 0), stop=(i == NWARM - 1))

            # ---- input DMAs (all up-front; tiles are per-aspect) ------
            wvs = cp.tile([128, DC, H], BF16, tag="wvs")
            nc.sync.dma_start(out=wvs.rearrange("p c h -> p (c h)"),
                              in_=wvpk.ap())
            cpf = cp.tile([128, CF_W], F32, tag="cpf")
            nc.scalar.dma_start(out=cpf, in_=cpackf.ap())
            cpb = cp.tile([128, CB_W], BF16, tag="cpb")
            g8 = cp.tile([128, DC, GW], F8, tag="g8")
            rows_bf = []
            for i in range(2):
                rb = cp.tile([97, N], BF16, tag=f"rows{i}")
                rows_bf.append(rb)

            # small consts first on the lightly-loaded rings so aspect-0
            # stream data monopolizes the queues right after
            nc.scalar.dma_start(out=cpb, in_=cpackb.ap())
            nc.gpsimd.dma_start(
                out=g8.rearrange("p c g -> p (c g)"), in_=gal8.ap())
            for rb in rows_bf:
                nc.gpsimd.dma_start(out=rb[96:97, :], in_=ngrow.ap())

            xx_t, xt_t, dp_t = {}, {}, {}
            for a in range(A):
                xx = xp.tile([128, DC, N], BF16, tag="xx")
                if a == 0:  # quarters for earliest first-chunk arrival
                    for i in range(4):
                        eng = nc.scalar if i % 2 == 0 else nc.sync
                        eng.dma_start(out=xx[:, 2 * i:2 * i + 2],
                                      in_=xs.ap()[a, :, 0, 2 * i:2 * i + 2])
                else:
                    nc.sync.dma_start(out=xx[:, 0:DC // 2],
                                      in_=xs.ap()[a, :, 0, 0:DC // 2])
                    nc.scalar.dma_start(out=xx[:, DC // 2:DC],
                                        in_=xs.ap()[a, :, 0, DC // 2:DC])
                dpa = xp.tile([128, DC, N], F8, tag="dp")
                nc.gpsimd.dma_start(out=dpa, in_=dp8.ap()[a])
                xt_ = xp.tile([128, DC, N], BF16, tag="xt")
                nc.sync.dma_start(out=xt_[:, 0:DC // 2],
                                  in_=xs.ap()[a, :, 1, 0:DC // 2])
                nc.scalar.dma_start(out=xt_[:, DC // 2:DC],
                                    in_=xs.ap()[a, :, 1, DC // 2:DC])
                xx_t[a], xt_t[a], dp_t[a] = xx, xt_, dpa

            # ---- constant views ---------------------------------------
            wv_v = wvs
            gall_v = cpb[:, CB_GALL:CB_GALL + DC * GW].rearrange(
                "p (c g) -> p c g", c=DC)
            combo_m = cpb[0:97, CB_COMBO:CB_COMBO + 3]
            bv_c = cpf[:, CF_BV:CF_BV + 1]
            bias_all = cpf[0:3, CF_BA:CF_BA + A]
            combw3 = cpf[0:3, CF_CW:CF_CW + 1]

            ones3r = cp.tile([3, 128], BF16, tag="ones3r")
            nc.vector.memset(ones3r, 1.0)
            attz = cp.tile([H, A], F32, tag="attz")

            # ---- per-aspect stream work -------------------------------
            # Uniform-config matmul chains run at 216ns/instr; alternating
            # PE tile configs cost ~+105ns each, so chains stay contiguous.
            # Aspects 0-2: V_W first (only needs the X plane, which lands
            # first).  Last aspect: rows first so the softmax chain hides
            # completely under the V_W/V_T chains and the tail is short.
            for a in range(A):
                xx, xt_, da = xx_t[a], xt_t[a], dp_t[a]
                win = slice(4 * a, 4 * a + 32)
                last = a == A - 1

                ps_vw = vps.tile([H, N], F32, tag="v")
                ps_rows = rps.tile([128, N], F32, tag="rows")
                vvw = wp.tile([H, N], BF16, tag="vvw")
                rb = rows_bf[a % 2]

                def vw_chain(combo_at=None):
                    for c in range(DC):
                        nc.tensor.matmul(ps_vw, lhsT=wv_v[:, c, :],
                                         rhs=xx[:, c, :], start=(c == 0),
                                         stop=(c == DC - 1))
                        if c == combo_at:
                            combo_exp()
                    nc.scalar.activation(vvw, ps_vw, AF.Identity, bias=bv_c)

                def rows_chains():
                    for c in range(DC):
                        nc.tensor.matmul(ps_rows[32:64, :],
                                         lhsT=gall_v[:, c, win],
                                         rhs=xx[:, c, :], start=(c == 0),
                                         stop=(c == DC - 1),
                                         tile_position=(0, 32))
                    # Dp rows: fp8 DoubleRow, 2 K-chunks per instruction
                    for c2 in range(DC // 2):
                        nc.tensor.matmul(ps_rows[0:32, :],
                                         lhsT=g8[:, 2 * c2:2 * c2 + 2, win],
                                         rhs=da[:, 2 * c2:2 * c2 + 2, :],
                                         start=(c2 == 0),
                                         stop=(c2 == DC // 2 - 1),
                                         perf_mode=DR, tile_position=(0, 0))
                    for c in range(DC):
                        nc.tensor.matmul(ps_rows[64:96, :],
                                         lhsT=gall_v[:, c, win],
                                         rhs=xt_[:, c, :], start=(c == 0),
                                         stop=(c == DC - 1),
                                         tile_position=(0, 64))
                    # rows -> bf16 (partition 96 = persistent neg row)
                    nc.scalar.activation(rb[0:96, :], ps_rows[0:96, :],
                                         AF.Identity)

                ps_combo = sps.tile([3, N], F32, tag="s")
                e3 = wp.tile([3, N], BF16, tag="e3")
                z3 = wp.tile([3, 1], F32, tag="z3")

                def combo_exp():
                    nc.tensor.matmul(ps_combo, lhsT=combo_m, rhs=rb,
                                     start=True, stop=True)
                    nc.scalar.activation(e3, ps_combo, AF.Exp,
                                         bias=bias_all[:, a:a + 1],
                                         scale=inv_s, accum_out=z3)

                ps_vt = vps.tile([H, N], F32, tag="v")

                def vt_chain(combo_at=None):
                    for c in range(DC):
                        nc.tensor.matmul(ps_vt, lhsT=wv_v[:, c, :],
                                         rhs=xt_[:, c, :], start=(c == 0),
                                         stop=(c == DC - 1))
                        if c == combo_at:
                            combo_exp()

                if last:
                    rows_chains()
                    vw_chain(combo_at=3)
                    vt_chain()
                else:
                    vw_chain()
                    rows_chains()
                    vt_chain(combo_at=2)

                rz = wp.tile([3, 1], F32, tag="rz")
                nc.vector.reciprocal(rz, z3)
                alpha = wp.tile([3, 1], F32, tag="alpha")
                nc.vector.tensor_mul(alpha, rz, combw3)
                arep = wp.tile([3, H], BF16, tag="arep")
                nc.vector.tensor_scalar_mul(arep, ones3r, alpha)
                # pprod = (VT+bv)*(VW+bv) right after the VT chain, then
                # attMM, then one multiply-accumulate into attz[:, a]
                pprod = wp.tile([H, N], BF16, tag="pprod")
                nc.vector.scalar_tensor_tensor(
                    pprod, ps_vt, bv_c, vvw, op0=OP.add, op1=OP.mult)
                ps_att = sps.tile([H, N], F32, tag="s")
                nc.tensor.matmul(ps_att, lhsT=arep, rhs=e3,
                                 start=True, stop=True)
                junk = wp.tile([H, N], BF16, tag="junk")
                nc.vector.scalar_tensor_tensor(
                    junk, ps_att, 1.0, pprod, op0=OP.mult, op1=OP.mult,
                    accum_out=attz[:, a:a + 1])

            nc.sync.dma_start(out=out.ap(), in_=attz)

    nc.compile()
    return nc


def _host_precompute(f):
    """All aspect-level math in fp64 on host -> per-core const packs."""
    S = SCALE
    Wq = f["Wq"].astype(np.float64)
    Wk = f["Wk"].astype(np.float64)
    TA = f["trans_W"][:H].astype(np.float64)   # [H, H]
    TB = f["trans_W"][H:].astype(np.float64)
    W1a = f["W1_W"][:H].astype(np.float64)
    W1b = f["W1_W"][H:].astype(np.float64)
    T1 = f["T1"].astype(np.float64)
    bq, bk = f["bq"].astype(np.float64), f["bk"].astype(np.float64)
    W1_b = f["W1_b"].astype(np.float64)
    trans_b = f["trans_b"].astype(np.float64)

    asp = f["aspect_feature"].astype(np.float64)          # [B, A, D]
    q = asp @ Wq + bq                                     # [B, A, H]
    u = np.einsum("kh,bah->bak", TA, q)
    v = np.einsum("jh,bah->baj", TB, q)
    y = np.einsum("kj,baj->bak", W1b, v)
    a3 = np.einsum("ij,bai->baj", W1a, q)
    QT = np.einsum("bai,ijk->bajk", q, T1)
    w = np.einsum("bajk,baj->bak", QT, v)
    G = np.stack([np.einsum("dh,bah->bad", Wk, t) for t in (q, w, y, u)],
                 axis=-1)                                 # [B, A, D, 4]
    Cb = q @ bk                                           # [B, A]
    Cdw = ((u + w + y) @ bk + ((a3 + W1_b) * v).sum(-1) + q @ trans_b)
    bias_all = np.stack([Cb, Cb, Cdw], axis=1) / S        # [B, 3, A]
    return G, bias_all


def _prep_inputs(inputs):
    f = {k: np.asarray(v, dtype=np.float32) for k, v in inputs.items()}
    G, bias_all = _host_precompute(f)

    wvpk = np.transpose(
        f["Wv"].reshape(DC, 128, H), (1, 0, 2)).reshape(128, DC * H)
    cpackb = np.zeros((128, CB_W), np.float32)
    # quadrants: Dp@0, X@32, T@64, neg@96.
    # combo rows: ch0(TW): st@64+neg; ch1(Wi): sxq@32+neg;
    # ch2(DW): sxw@33, sxy@34, sd@3 (fp8 panel scaled by G8S), neg
    cpackb[64, CB_COMBO + 0] = 1.0
    cpackb[96, CB_COMBO + 0] = 1.0
    cpackb[32, CB_COMBO + 1] = 1.0
    cpackb[96, CB_COMBO + 1] = 1.0
    cpackb[33, CB_COMBO + 2] = 1.0
    cpackb[34, CB_COMBO + 2] = 1.0
    cpackb[3, CB_COMBO + 2] = 1.0 / G8S
    cpackb[96, CB_COMBO + 2] = 1.0

    in_maps = []
    for b in range(NCORES):
        # gall panel [128, DC, 48]: cols 4a+s = G[b, a, :, s]
        gp = np.zeros((D, GW), np.float64)
        for a in range(A):
            gp[:, 4 * a:4 * a + 4] = G[b, a]
        gp = gp.reshape(DC, 128, GW).transpose(1, 0, 2)   # [128, DC, GW]
        cb = cpackb.copy()
        cb[:, CB_GALL:CB_GALL + DC * GW] = gp.reshape(128, DC * GW)

        cf = np.zeros((128, CF_W), np.float32)
        cf[:, CF_BV] = f["bv"]
        cf[0:3, CF_BA:CF_BA + A] = bias_all[b]
        cf[0:3, CF_CW] = f["comb_w"]

        m = {
            "cpackb": cb.astype(BF),
            "wvpk": wvpk.astype(BF),
            "cpackf": cf,
            "gal8": np.clip(gp * G8S, -448, 448).astype(E4).reshape(
                128, DC * GW),
            "ngrow": (-1e30 * (1.0 - f["fmask"][b]))[None, :].astype(BF),
        }
        xst = np.stack([f["feature"][b], f["all_type_feature"][b]], axis=1)
        # [A, 2, N, D] -> [A, 128(p), 2(s), DC(c), N]
        m["xs"] = np.ascontiguousarray(
            xst.transpose(0, 1, 3, 2).reshape(A, 2, DC, 128, N)
               .transpose(0, 3, 1, 2, 4)).astype(BF)
        dpt = f["dep_feature"][b].transpose(0, 2, 1).reshape(A, DC, 128, N)
        m["dp8"] = np.clip(np.ascontiguousarray(dpt.transpose(0, 2, 1, 3)),
                           -240, 240).astype(E4)
        in_maps.append(m)
    return in_maps


def _install_ntff_shim():
    """Provide antenv.axon_hooks (absent in this image) so trace=True can
    drive NTFF capture through libaxon_pjrt.so."""
    if "antenv.axon_hooks" in sys.modules:
        return
    import antenv

    mod = types.ModuleType("antenv.axon_hooks")
    mod._hook = None
    mod.set_axon_ntff_profile_hook = lambda h: setattr(mod, "_hook", h)
    mod.get_axon_ntff_profile_hook = lambda: mod._hook
    sys.modules["antenv.axon_hooks"] = mod
    antenv.axon_hooks = mod

    so_path = "/opt/axon/libaxon_pjrt.so"
    try:
        lib = ctypes.CDLL(so_path)
    except OSError:
        return
    if not hasattr(lib, "axon_start_nrt_profile"):
        return
    lib.axon_start_nrt_profile.argtypes = [ctypes.POINTER(ctypes.c_int64),
                                           ctypes.c_size_t]
    lib.axon_start_nrt_profile.restype = ctypes.c_int64
    lib.axon_stop_nrt_profile.argtypes = [ctypes.c_char_p]
    lib.axon_stop_nrt_profile.restype = ctypes.c_int64

    @contextlib.contextmanager
    def _hook(output_dir, device_ids):
        import jax

        jax.devices()
        if device_ids:
            ids = (ctypes.c_int64 * len(device_ids))(*device_ids)
            rc = lib.axon_start_nrt_profile(ids, len(device_ids))
        else:
            rc = lib.axon_start_nrt_profile(None, 0)
        if rc != 0:
            raise RuntimeError(f"axon_start_nrt_profile rc={rc}")
        try:
            yield
        finally:
            n = lib.axon_stop_nrt_profile(str(output_dir).encode())
            print(f"profile: {n} file(s) written to {output_dir}")

    mod.set_axon_ntff_profile_hook(_hook)


def kernel(feature, dep_feature, aspect_feature, all_type_feature, fmask,
           Wq, bq, Wk, bk, Wv, bv, trans_W, trans_b, T1, W1_W, W1_b, comb_w,
           _profile=False, _tmpdir=None):
    global LAST_RESULTS
    inputs = dict(feature=feature, dep_feature=dep_feature,
                  aspect_feature=aspect_feature,
                  all_type_feature=all_type_feature, fmask=fmask, Wq=Wq,
                  bq=bq, Wk=Wk, bk=bk, Wv=Wv, bv=bv, trans_W=trans_W,
                  trans_b=trans_b, T1=T1, W1_W=W1_W, W1_b=W1_b,
                  comb_w=comb_w)
    nc = _build()
    in_maps = _prep_inputs(inputs)
    if _profile:
        _install_ntff_shim()
    res = run_bass_kernel_spmd(nc, in_maps, list(range(NCORES)),
                               trace=_profile, tmpdir=_tmpdir)
    LAST_RESULTS = res
    full = np.stack([res.results[c]["out"].T for c in range(NCORES)])
    return full.astype(np.float32)
